# revision 33
# baseline (speedup 1.0000x reference)
"""LowRankMixtureDensityNetwork loss on 8 Trainium2 NeuronCores.

Data-parallel over the batch (1024 rows/core), MLP weights replicated.
BatchNorm (training mode) statistics are allreduced across cores per layer.

Tail strategy (v3): the output layer and mixture tail run FEATURE-on-
partition. Output feature tiles are [(k4, d32), b]; per-partition biases
are applied on ACT during psum evacuation. The bordered low-rank rows
A_r = (fac_r + b) * exp(-diag/2) and u = -(data - mu) * exp(-diag/2) are
built elementwise; Gram pair products run as full-batch diagonal slabs on
DVE/GpSimd, and the d-reduction is a TensorE matmul against one of eight
block-diagonal ones stationaries, scattering each pair's [4k, b] fold into
32-partition groups of shared PSUM tiles (tile_position column quadrants;
groups issue strictly sequentially). Small TensorE transposes flip the
capacitance entries back to batch-on-partition for the bordered 9x9 LDL,
whose slots are diagonal-major compact with the +I folded into the pivot
extraction (column accesses are per-plane).
"""
import contextlib

import numpy as np

import concourse.bass as bass
import concourse.tile as tile
from concourse import mybir
import bass_rust

F32 = mybir.dt.float32
FP8 = mybir.dt.float8e4
BF16 = mybir.dt.bfloat16
AF = mybir.ActivationFunctionType
ALU = mybir.AluOpType

# problem constants
DIM, K, RANK = 32, 16, 8
CTX, H, NL, B = 128, 512, 4, 8192
OUT = K + DIM * K + (DIM + DIM * RANK) * K          # 5136
N_CORES = 8
BL = B // N_CORES                                    # 1024 rows per core
NR = RANK + 1                                        # 9 (bordered system)
NT = 41                                              # feature tiles (40 + w)
LOG2PI = float(np.log(2.0 * np.pi))

# diagonal-major compact slot base: pair (a, b), a<=b -> DIAGBASE[b-a] + a
DIAGBASE = [0, 9, 17, 24, 30, 35, 39, 42, 44]        # 45 pair slots
NSLOT = 46                                           # +1 for ldraw
NU_F1 = 32

# fold positions issue in diagonal-major order (0..44), then ldraw (45):
# psum groups are strictly sequential. Position 45 (ldraw) gets its own
# 32-aligned group (F2 rows 64..67) so its ACT evac is quadrant-aligned.
_GFIRST = {0: 0, 1: 8, 2: 16, 3: 24, 4: 32, 5: 40, 6: 45}
_GLAST = {0: 7, 1: 15, 2: 23, 3: 31, 4: 39, 5: 44, 6: 45}

# engine split knobs
GRAM_GPS_O = (5, 6, 7, 8)      # Gram diagonals whose products run on GpSimd
ABUILD_GPS_R = (5, 6, 7)       # A-build rows multiplied on GpSimd

# ------------------------------------------------------------- walrus quirks

_ctr = [0]


def _split_multi_waits(nc, max_waits=1):
    """walrus in this container rejects >1 sync wait per instruction; hoist
    excess waits onto same-engine NOPs placed just before the instruction."""
    n_split = 0
    for f in nc.m.functions:
        for bb in f.blocks:
            insts = bb.instructions
            out = []
            changed = False
            for inst in insts:
                si = inst.sync_info
                waits = list(si.on_wait) if si is not None else []
                if len(waits) > max_waits:
                    for w in waits[:-max_waits]:
                        _ctr[0] += 1
                        nop = mybir.InstNoOp(
                            name=f"WSPLIT-{_ctr[0]}",
                            engine=inst.engine,
                            ins=[],
                            outs=[],
                            sync_info=mybir.SyncInfo(on_wait=[w], on_update=[]),
                        )
                        out.append(nop)
                    inst.sync_info = mybir.SyncInfo(
                        on_wait=waits[-max_waits:], on_update=list(si.on_update)
                    )
                    changed = True
                    n_split += 1
                out.append(inst)
            if changed:
                bb.instructions = out
    return n_split


def _patched_drain_and_barrier(self, tick_clock, wait_clock):
    nc = self.nc
    probe = nc.sync.nop()
    wait_clock.add_sem_waits(
        probe.ins, bass_rust.ScopedClock({None: tick_clock.global_clock})
    )
    si = probe.ins.sync_info
    waits = list(si.on_wait) if si is not None else []
    if len(waits) > 1:
        probe.ins.sync_info = mybir.SyncInfo(on_wait=waits[:1], on_update=[])
        for w in waits[1:]:
            extra = nc.sync.nop()
            extra.ins.sync_info = mybir.SyncInfo(on_wait=[w], on_update=[])
    nc.sync.drain()

    nc.all_engine_barrier()
    assert self.sems is not None
    popped = nc._tile_sem_poison_stack.pop()
    assert popped is self._sem_poison
    nc.clear_and_free_semaphores(list(self.sems.allocated().values()))
    nc.all_engine_barrier()


tile.TileContext._drain_and_barrier = _patched_drain_and_barrier


def _bc_mid(ap, n):
    """[P, inner] AP -> [P, n, inner] with a stride-0 middle dim"""
    return bass.AP(tensor=ap.tensor, offset=ap.offset,
                   ap=[ap.ap[0], [0, n], ap.ap[-1]])


def _bc_inner(ap, k):
    """[P, n] AP -> [P, n, k] with a stride-0 inner dim"""
    return bass.AP(tensor=ap.tensor, offset=ap.offset,
                   ap=[ap.ap[0], ap.ap[-1], [0, k]])


# ----------------------------------------------------------------- program


def build_program(split=True):
    nc = bass.Bass("TRN2", num_devices=N_CORES)

    ctxT = nc.dram_tensor("ctxT", [CTX, BL], BF16, kind="ExternalInput")
    dataT = nc.dram_tensor("dataT", [DIM, BL], F32, kind="ExternalInput")
    w0t = nc.dram_tensor("w0t", [CTX, H], BF16, kind="ExternalInput")
    wht = nc.dram_tensor("wht", [NL - 1, H, H], BF16, kind="ExternalInput")
    woutt = nc.dram_tensor("woutt", [128, 2, 2, 9 * 128], BF16,
                           kind="ExternalInput")
    wout8 = nc.dram_tensor("wout8", [128, 2, 2, 32 * 128],
                           mybir.dt.float8e4, kind="ExternalInput")
    biasft = nc.dram_tensor("biasft", [128, 45], F32, kind="ExternalInput")
    # per-feature vectors packed [128, 4hc, 12]:
    #   0:b0 1:g0 2:be0, then per hidden l (0..2): 3+3l:bh, 4+3l:gh, 5+3l:beh
    vecs = nc.dram_tensor("vecs", [128, 4, 12], F32, kind="ExternalInput")
    pat8 = nc.dram_tensor("pat8", [128, 8, 32], BF16, kind="ExternalInput")
    ident = nc.dram_tensor("ident", [128, 128], BF16, kind="ExternalInput")
    eye16 = nc.dram_tensor("eye16", [16, 16], F32, kind="ExternalInput")
    yout = nc.dram_tensor("yout", [1, 1], F32, kind="ExternalOutput")

    with tile.TileContext(nc) as tc:
        _body(nc, tc, ctxT, dataT, w0t, wht, woutt, wout8, biasft, vecs,
              pat8, ident, eye16, yout)
    if split:
        _split_multi_waits(nc)
    return nc


def _mlp(nc, tc, sb1, dram, ctxT, w0t, wht, vecs):
    """feature-on-partition MLP with cross-core BN; returns u3p (bf16)."""
    ctx = contextlib.ExitStack()
    sbm = ctx.enter_context(tc.tile_pool(name="mlpwork", bufs=2))
    sbu = ctx.enter_context(tc.tile_pool(name="uacts", bufs=2))
    sbe = ctx.enter_context(tc.tile_pool(name="elu", bufs=3))
    ps = ctx.enter_context(tc.tile_pool(name="psm", bufs=1, space="PSUM"))

    t_ctx = sbm.tile([128, BL], BF16, name="t_ctx", tag="t_ctx", bufs=1)
    nc.sync.dma_start(out=t_ctx[:], in_=ctxT[:])
    t_w0 = sbm.tile([128, H], BF16, name="t_w0", tag="t_w0", bufs=1)
    nc.sync.dma_start(out=t_w0[:], in_=w0t[:])
    t_wh = sbm.tile([128, NL - 1, 2, 2, H], BF16, name="t_wh", tag="t_wh",
                    bufs=1)
    nc.sync.dma_start(
        out=t_wh[:], in_=wht.rearrange("l (r i p) m -> p l r i m", r=2, i=2))
    t_vec = sbm.tile([128, 4, 12], F32, name="t_vec", tag="t_vec", bufs=1)
    nc.sync.dma_start(out=t_vec[:], in_=vecs[:])
    eps_t = sbm.tile([128, 1], F32, name="eps_t", tag="eps_t", bufs=1)
    nc.vector.memset(eps_t[:], 1e-5)

    # collective warmup (absorb first-collective latency)
    cwu_in = dram.tile([128, 1], F32, name="cwu_in")
    cwu_out = dram.tile([128, 1], F32, name="cwu_out")
    t_junk = sbm.tile([128, 1], F32, name="t_junk", tag="t_junk", bufs=1)
    nc.vector.memset(t_junk[:], 0.0)
    nc.sync.dma_start(out=cwu_in[:], in_=t_junk[:])
    nc.gpsimd.collective_compute(
        "AllReduce", ALU.add, replica_groups=[list(range(N_CORES))],
        ins=[cwu_in[:].opt()], outs=[cwu_out[:].opt()],
    )
    nc.gpsimd.collective_compute(
        "AllReduce", ALU.add, replica_groups=[list(range(N_CORES))],
        ins=[cwu_out[:].opt()], outs=[cwu_in[:].opt()],
    )
    t_junk2 = sbm.tile([128, 1], F32, name="t_junk2", tag="t_junk2", bufs=1)
    nc.gpsimd.dma_start(out=t_junk2[:], in_=cwu_in[:])

    u_prev = None
    u3p = None
    wfold = None
    beff = None

    for layer in range(NL):
        u_cur = sbu.tile([128, 4, BL], BF16, name=f"u{layer}", tag="u")
        nkc = 1 if layer == 0 else 4
        for hc in range(4):
            if layer == 0:
                bcol = t_vec[:, hc, 0:1]
            else:
                bcol = beff[:, hc:hc + 1]
            for bcc in range(2):
                bs = bcc * 512
                psum = ps.tile([128, 512], F32, name="zp", tag="z", bufs=3)
                if layer == 0:
                    nc.tensor.matmul(psum[:],
                                     lhsT=t_w0[:, hc * 128:(hc + 1) * 128],
                                     rhs=t_ctx[:, bs:bs + 512],
                                     start=True, stop=True)
                else:
                    for kc in range(4):
                        nc.tensor.matmul(
                            psum[:],
                            lhsT=wfold[:, kc // 2, kc % 2,
                                       hc * 128:(hc + 1) * 128],
                            rhs=u_prev[:, kc, bs:bs + 512],
                            start=(kc == 0), stop=(kc == 3))
                # ELU: u = min(exp(z+b) - 1, relu(z+b))
                e_t = sbe.tile([128, 512], F32, name="elu_e", tag="elu_e")
                nc.scalar.activation(e_t[:], psum[:], AF.Exp, bias=bcol)
                r_t = sbe.tile([128, 512], BF16, name="elu_r", tag="elu_r")
                nc.scalar.activation(r_t[:], psum[:], AF.Relu, bias=bcol)
                nc.vector.scalar_tensor_tensor(
                    u_cur[:, hc, bs:bs + 512], e_t[:], -1.0, r_t[:],
                    op0=ALU.add, op1=ALU.min)

        # ---- batch-norm stats (local) -> allreduce -> affine params
        stats = sbm.tile([128, 4, 2, 6], F32, name="bns", tag="bns")
        for hc in range(4):
            for half in range(2):
                nc.vector.bn_stats(
                    out=stats[:, hc, half, :],
                    in_=u_cur[:, hc, half * 512:(half + 1) * 512])
        mv = sbm.tile([128, 4, 2], F32, name="bnmv", tag="bnmv")
        for hc in range(4):
            nc.vector.bn_aggr(out=mv[:, hc, :], in_=stats[:, hc, :, :])
        pack = sbm.tile([128, 8], F32, name="bnp", tag="bnp")
        mm = mv[:, :, 0:1].rearrange("p h one -> p (h one)")
        vv = mv[:, :, 1:2].rearrange("p h one -> p (h one)")
        nc.vector.tensor_scalar_mul(pack[:, 0:4], mm, float(BL))
        msq = sbm.tile([128, 4], F32, name="bmsq", tag="bmsq")
        nc.vector.tensor_tensor(msq[:], mm, mm, op=ALU.mult)
        s2s = sbm.tile([128, 4], F32, name="bs2", tag="bs2")
        nc.vector.tensor_tensor(s2s[:], vv, msq[:], op=ALU.add)
        nc.vector.tensor_scalar_mul(pack[:, 4:8], s2s[:], float(BL))

        ar_in = dram.tile([128, 8], F32, name=f"arin{layer}")
        ar_out = dram.tile([128, 8], F32, name=f"arout{layer}")
        nc.sync.dma_start(out=ar_in[:], in_=pack[:])
        nc.gpsimd.collective_compute(
            "AllReduce", ALU.add, replica_groups=[list(range(N_CORES))],
            ins=[ar_in[:].opt()], outs=[ar_out[:].opt()],
        )
        warm = ps.tile([128, 512], F32, name="warm", tag="warm", bufs=1)
        for _ in range(22):
            nc.tensor.matmul(warm[:], lhsT=t_w0[:, 0:128],
                             rhs=t_ctx[:, 0:512], start=True, stop=True)
        red = sbm.tile([128, 8], F32, name="bnr", tag="bnr")
        nc.gpsimd.dma_start(out=red[:], in_=ar_out[:])

        iv = 0 if layer == 0 else 3 * (layer - 1) + 3
        g_col = t_vec[:, :, iv + 1]
        be_col = t_vec[:, :, iv + 2]
        m_t = sbm.tile([128, 4], F32, name="bnm", tag="bnm")
        nc.vector.tensor_scalar_mul(m_t[:], red[:, 0:4], 1.0 / B)
        msq2 = sbm.tile([128, 4], F32, name="bnm2", tag="bnm2")
        nc.vector.tensor_tensor(msq2[:], m_t[:], m_t[:], op=ALU.mult)
        var_t = sbm.tile([128, 4], F32, name="bnv", tag="bnv")
        nc.vector.scalar_tensor_tensor(
            var_t[:], red[:, 4:8], 1.0 / B, msq2[:],
            op0=ALU.mult, op1=ALU.subtract)
        # a = g * rsqrt(var+eps) = g * exp(-0.5*ln(var+eps))
        lnv = sbm.tile([128, 4], F32, name="bnl", tag="bnl")
        nc.scalar.activation(lnv[:], var_t[:], AF.Ln, bias=eps_t[:])
        rsq = sbm.tile([128, 4], F32, name="bnq", tag="bnq")
        nc.scalar.activation(rsq[:], lnv[:], AF.Exp, scale=-0.5)
        a_t = sbm.tile([128, 4], F32, name="bna", tag="bna")
        nc.vector.tensor_tensor(a_t[:], g_col, rsq[:], op=ALU.mult)
        ma = sbm.tile([128, 4], F32, name="bnma", tag="bnma")
        nc.vector.tensor_tensor(ma[:], m_t[:], a_t[:], op=ALU.mult)
        c_t = sbm.tile([128, 4], F32, name="bnc", tag="bnc")
        nc.vector.tensor_tensor(c_t[:], be_col, ma[:], op=ALU.subtract)

        if layer < NL - 1:
            # fold affine into next layer: W' = WhT * a (per contraction row)
            wfold = sbm.tile([128, 2, 2, H], BF16, name="wf", tag="wf")
            for kc in range(4):
                nc.vector.tensor_scalar_mul(
                    wfold[:, kc // 2, kc % 2, :],
                    t_wh[:, layer, kc // 2, kc % 2, :], a_t[:, kc:kc + 1])
            # bias: z_{l+1} = W'u + (Wh[layer] @ c + b_{l+1})
            c_bf = sbm.tile([128, 4], BF16, name="cbf", tag="cbf")
            nc.vector.tensor_copy(c_bf[:], c_t[:])
            beff = sbm.tile([128, 4], F32, name="beff", tag="beff")
            b_next = t_vec[:, :, 3 * layer + 3]
            for mc in range(4):
                pb = ps.tile([128, 1], F32, name="pbias", tag="pbias", bufs=1)
                for kc in range(4):
                    nc.tensor.matmul(
                        pb[:],
                        lhsT=t_wh[:, layer, kc // 2, kc % 2,
                                  mc * 128:(mc + 1) * 128],
                        rhs=c_bf[:, kc:kc + 1],
                        start=(kc == 0), stop=(kc == 3))
                nc.scalar.activation(
                    beff[:, mc:mc + 1], pb[:], AF.Identity,
                    bias=b_next[:, mc:mc + 1])
            u_prev = u_cur
        else:
            # BN3 applied directly on u (Wout stays raw)
            u3p = sb1.tile([128, 4, BL], BF16, name="u3p")
            u3p8 = sb1.tile([128, 4, BL], FP8, name="u3p8")
            for hc in range(4):
                nc.scalar.activation(
                    u3p[:, hc, :], u_cur[:, hc, :], AF.Identity,
                    bias=c_t[:, hc:hc + 1], scale=a_t[:, hc:hc + 1])
                nc.scalar.activation(
                    u3p8[:, hc, :], u_cur[:, hc, :], AF.Identity,
                    bias=c_t[:, hc:hc + 1], scale=a_t[:, hc:hc + 1])

    ctx.close()
    return u3p, u3p8


def _body(nc, tc, ctxT, dataT, w0t, wht, woutt, wout8, biasft, vecs,
          pat8, ident, eye16, yout):
    ctx = contextlib.ExitStack()
    sb1 = ctx.enter_context(tc.tile_pool(name="persist", bufs=1))
    dram = ctx.enter_context(tc.tile_pool(name="dram", bufs=1, space="DRAM"))

    t_wo = sb1.tile([128, 2, 2, 9 * 128], BF16, name="t_wo")
    nc.sync.dma_start(out=t_wo[:], in_=woutt[:])
    t_wo8 = sb1.tile([128, 2, 2, 32 * 128], FP8, name="t_wo8")
    nc.sync.dma_start(out=t_wo8[:], in_=wout8[:])
    t_bias = sb1.tile([128, 45], F32, name="t_bias")
    nc.sync.dma_start(out=t_bias[:], in_=biasft[:])
    t_pat = sb1.tile([128, 8, 32], BF16, name="t_pat")
    nc.sync.dma_start(out=t_pat[:], in_=pat8[:])
    t_id = sb1.tile([128, 128], BF16, name="t_id")
    nc.sync.dma_start(out=t_id[:], in_=ident[:])
    t_eye16 = sb1.tile([16, 16], F32, name="t_eye16")
    nc.sync.dma_start(out=t_eye16[:], in_=eye16[:])
    dataRep = sb1.tile([128, BL], F32, name="dataRep")
    for g in range(4):
        nc.sync.dma_start(out=dataRep[32 * g:32 * (g + 1), :], in_=dataT[:])

    u3p, u3p8 = _mlp(nc, tc, sb1, dram, ctxT, w0t, wht, vecs)
    ps = ctx.enter_context(tc.tile_pool(name="ps", bufs=1, space="PSUM"))

    # persistent tail state
    capG = sb1.tile([128, NSLOT, 128], BF16, name="capG")   # slot, (c8,k16)
    ldall = sb1.tile([128, 8, K], F32, name="ldall")
    wall = sb1.tile([128, 8, K], F32, name="wall")

    sbt = ctx.enter_context(tc.tile_pool(name="tail", bufs=2))

    # bias column helpers (feature-on-partition, per kg)
    def b_mu(kg):
        return t_bias[:, kg:kg + 1]

    def b_diag(kg):
        return t_bias[:, 4 + kg:5 + kg]

    def b_dhalf(kg):
        return t_bias[:, 8 + kg:9 + kg]

    def b_fac(kg, r):
        c = 12 + kg * 8 + r
        return t_bias[:, c:c + 1]

    b_w = t_bias[0:16, 44:45]

    state = {}

    def stage_A(kg, half):
        """output GEMMs + A-row builds for (kg, half) (PE/ACT/DVE/GPS)."""
        if half == 0:
            state[kg] = {
                "At": sbt.tile([128, NR, BL], BF16, name="At", tag="At",
                               bufs=2),
                "ldb": sbt.tile([128, BL], BF16, name="ldb", tag="ldb",
                                bufs=2),
            }
        At = state[kg]["At"]
        ldb_bf = state[kg]["ldb"]
        bs = half * 512

        def gemm(t):
            pg = ps.tile([128, 512], F32, name="pg", tag="gemm", bufs=2)
            for kc in range(4):
                nc.tensor.matmul(
                    pg[:], lhsT=t_wo[:, kc // 2, kc % 2,
                                     t * 128:(t + 1) * 128],
                    rhs=u3p[:, kc, bs:bs + 512],
                    start=(kc == 0), stop=(kc == 3))
            return pg

        # diag tile: s = exp(-(raw+b)/2), ldb = raw+b
        pg = gemm(4 + kg)
        s_bf = sbt.tile([128, 512], BF16, name="s_bf", tag="s_bf")
        nc.scalar.activation(s_bf[:], pg[:], AF.Exp,
                             bias=b_dhalf(kg), scale=-0.5)
        nc.scalar.activation(ldb_bf[:, bs:bs + 512], pg[:], AF.Identity,
                             bias=b_diag(kg))

        # mu tile: diffn = (mu+b) - data (negated diff; sign cancels)
        pg = gemm(kg)
        diffn = sbt.tile([128, 512], BF16, name="diffn", tag="diffn")
        nc.vector.scalar_tensor_tensor(
            diffn[:], pg[:], b_mu(kg), dataRep[:, bs:bs + 512],
            op0=ALU.add, op1=ALU.subtract)
        nc.vector.tensor_tensor(At[:, RANK, bs:bs + 512], diffn[:],
                                s_bf[:], op=ALU.mult)

        # A rows: fused (fac+b)*s from psum on DVE; some rows via
        # ACT evac + GpSimd multiply to spread engine load.
        # fac GEMMs run in fp8 DoubleRow (256-deep contraction).
        def gemm8(t):
            pg = ps.tile([128, 512], F32, name="pg", tag="gemm", bufs=2)
            for pr in range(2):
                nc.tensor.matmul(
                    pg[:], lhsT=t_wo8[:, pr, :, t * 128:(t + 1) * 128],
                    rhs=u3p8[:, 2 * pr:2 * pr + 2, bs:bs + 512],
                    start=(pr == 0), stop=(pr == 1),
                    perf_mode=mybir.MatmulPerfMode.DoubleRow)
            return pg

        for r in range(RANK):
            pg = gemm8(kg * 8 + r)
            if r in ABUILD_GPS_R:
                fbf = sbt.tile([128, 512], BF16, name="fbf", tag="fbf",
                               bufs=4)
                nc.scalar.activation(fbf[:], pg[:], AF.Identity,
                                     bias=b_fac(kg, r))
                nc.gpsimd.tensor_tensor(At[:, r, bs:bs + 512], fbf[:],
                                        s_bf[:], op=ALU.mult)
            else:
                nc.vector.scalar_tensor_tensor(
                    At[:, r, bs:bs + 512], pg[:], b_fac(kg, r), s_bf[:],
                    op0=ALU.add, op1=ALU.mult)

    def stage_BC(kg, h):
        """Gram products (DVE/GPS) + fold matmuls (PE) for (kg, half)."""
        At = state[kg]["At"]
        ldb_bf = state[kg]["ldb"]
        f1 = ps.tile([128, 512], F32, name=f"f1h{h}", tag=f"F1{h}", bufs=1)
        f2 = ps.tile([96, 512], F32, name=f"f2h{h}", tag=f"F2{h}", bufs=1)
        state[kg][f"f1h{h}"], state[kg][f"f2h{h}"] = f1, f2
        hs = 512 * h

        def fold(pos, mov):
            if pos == 45:
                g, p = 6, 0
                out, tp = f2[64:96, :], (0, 64)
            else:
                g, p = divmod(pos, 8)
                if pos < NU_F1:
                    out, tp = f1[32 * g:32 * g + 32, :], (0, 32 * g)
                else:
                    out = f2[32 * (g - 4):32 * (g - 4) + 32, :]
                    tp = (0, 32 * (g - 4))
            nc.tensor.matmul(out, lhsT=t_pat[:, p, :], rhs=mov,
                             start=(_GFIRST[g] == pos),
                             stop=(_GLAST[g] == pos), tile_position=tp)

        pos = 0
        for o in range(NR):
            n = NR - o
            P = sbt.tile([128, NR, 512], BF16, name="P", tag="P", bufs=4)
            eng = nc.gpsimd if o in GRAM_GPS_O else nc.vector
            if o < 2:  # split big slabs: let the first folds start earlier
                nsp = n // 2
                eng.tensor_tensor(P[:, 0:nsp, :], At[:, 0:nsp, hs:hs + 512],
                                  At[:, o:o + nsp, hs:hs + 512], op=ALU.mult)
                eng.tensor_tensor(P[:, nsp:n, :], At[:, nsp:n, hs:hs + 512],
                                  At[:, o + nsp:NR, hs:hs + 512], op=ALU.mult)
            else:
                eng.tensor_tensor(P[:, 0:n, :], At[:, 0:n, hs:hs + 512],
                                  At[:, o:NR, hs:hs + 512], op=ALU.mult)
            for a in range(n):
                fold(pos, P[:, a, :])
                pos += 1
        fold(45, ldb_bf[:, hs:hs + 512])

    def stage_D(kg):
        """fold psum evac + transposes + capG scatter for kg (ACT/PE)."""
        cf1 = sbt.tile([128, BL], BF16, name="cf1", tag="cf1")
        cf2 = sbt.tile([52, BL], BF16, name="cf2", tag="cf2")
        ldb32 = sbt.tile([4, BL], F32, name="ldb32", tag="ldb32")
        for h in range(2):
            hs = 512 * h
            f1, f2 = state[kg][f"f1h{h}"], state[kg][f"f2h{h}"]
            nc.scalar.copy(cf1[:, hs:hs + 512], f1[:])
            nc.scalar.copy(cf2[:, hs:hs + 512], f2[0:52, :])
            nc.scalar.copy(ldb32[:, hs:hs + 512], f2[64:68, :])
        for c in range(8):
            ioff = c * K + kg * 4
            pt1 = ps.tile([128, 128], BF16, name="pt1", tag="tr", bufs=2)
            nc.tensor.transpose(pt1[:], cf1[:, c * 128:(c + 1) * 128],
                                t_id[:])
            nc.scalar.copy(capG[:, 0:32, ioff:ioff + 4],
                           pt1[:].rearrange("p (u i) -> p u i", i=4))
            pt2f = ps.tile([128, 128], BF16, name="pt2", tag="tr", bufs=2)
            pt2 = pt2f[:, 0:52]
            nc.tensor.transpose(pt2, cf2[:, c * 128:(c + 1) * 128],
                                t_id[0:52, 0:52])
            nc.scalar.copy(capG[:, 32:45, ioff:ioff + 4],
                           pt2.rearrange("p (u i) -> p u i", i=4))
            pt3f = ps.tile([128, 16], F32, name="pt3", tag="tr", bufs=2)
            pt3 = pt3f[:, 0:4]
            nc.tensor.transpose(pt3, ldb32[:, c * 128:(c + 1) * 128],
                                t_eye16[0:4, 0:4])
            nc.scalar.copy(ldall[:, c, kg * 4:(kg + 1) * 4], pt3)
        del state[kg]

    def lse_k(src3d, nm):
        mx = sbt.tile([128, 8], F32, name=f"mx{nm}", tag=f"mx{nm}")
        nc.vector.tensor_reduce(out=mx[:], in_=src3d,
                                axis=mybir.AxisListType.X, op=ALU.max)
        zs = sbt.tile([128, 8, K], F32, name=f"zs{nm}", tag=f"zs{nm}")
        nc.vector.tensor_tensor(zs[:], src3d, _bc_inner(mx[:], K),
                                op=ALU.subtract)
        ez = sbt.tile([128, 8, K], F32, name=f"ez{nm}", tag=f"ez{nm}")
        nc.scalar.activation(ez[:], zs[:], AF.Exp)
        sez = sbt.tile([128, 8], F32, name=f"se{nm}", tag=f"se{nm}")
        nc.vector.tensor_reduce(out=sez[:], in_=ez[:],
                                axis=mybir.AxisListType.X, op=ALU.add)
        ls = sbt.tile([128, 8], F32, name=f"ls{nm}", tag=f"ls{nm}")
        nc.scalar.activation(ls[:], sez[:], AF.Ln)
        out = sbt.tile([128, 8], F32, name=f"lo{nm}", tag=f"lo{nm}")
        nc.vector.tensor_tensor(out[:], mx[:], ls[:], op=ALU.add)
        return out

    def emit_w():
        # w tile: logits to batch layout + its logsumexp (independent of
        # the LDL; emitted early to fill tail gaps and shorten the endgame)
        wsb = sbt.tile([16, BL], F32, name="wsb", tag="wsb", bufs=1)
        for half in range(2):
            bs = half * 512
            pgw = ps.tile([128, 512], F32, name="pgw", tag="gemm", bufs=2)
            for kc in range(4):
                nc.tensor.matmul(
                    pgw[:], lhsT=t_wo[:, kc // 2, kc % 2, 8 * 128:9 * 128],
                    rhs=u3p[:, kc, bs:bs + 512],
                    start=(kc == 0), stop=(kc == 3))
            nc.scalar.activation(wsb[:, bs:bs + 512], pgw[0:16, :],
                                 AF.Identity, bias=b_w)
        for c in range(8):
            ptw = ps.tile([128, 16], F32, name="ptw", tag="tr", bufs=2)
            nc.tensor.transpose(ptw[:], wsb[:, c * 128:(c + 1) * 128],
                                t_eye16[:])
            nc.scalar.copy(wall[:, c, :], ptw[:])
        return lse_k(wall[:], "w")

    # software pipeline at half-batch granularity: the PE streams the next
    # half's GEMMs while DVE/GPS run the current half's Gram products.
    stage_A(0, 0)
    stage_A(0, 1)
    stage_BC(0, 0)
    lpw = emit_w()
    stage_A(1, 0)
    stage_BC(0, 1)
    stage_A(1, 1)
    stage_D(0)
    stage_BC(1, 0)
    stage_A(2, 0)
    stage_BC(1, 1)
    stage_A(2, 1)
    stage_D(1)
    stage_BC(2, 0)
    stage_A(3, 0)
    stage_BC(2, 1)
    stage_A(3, 1)
    stage_D(2)
    stage_BC(3, 0)
    stage_BC(3, 1)
    stage_D(3)

    # ---------------- bordered slab LDL over [128, 128] planes
    # V lives in capG diagonal-major compact slots (entry (a,b) at
    # DIAGBASE[b-a]+a); +I folded into the pivot extraction. Column access
    # is per-plane (non-constant slot stride across diagonals).
    # L is compact r-major: (i,p) at rs0(p)+(i-p)
    def rs0(r):
        return r * NR - r * (r - 1) // 2

    BK = 128
    Lbf = sb1.tile([128, 45, BK], BF16, name="Lbf")
    pivd = sb1.tile([128, NR, BK], F32, name="pivd")
    ldt = sb1.tile([128, BK], F32, name="ldt")
    nc.vector.tensor_copy(ldt[:], ldall[:].rearrange("p c k -> p (c k)"))

    def vplane(a, b):
        """cap entry (a, b) with a<=b -> [128, BK]"""
        return capG[:, DIAGBASE[b - a] + a, :]

    def lcol(p, i0, i1):
        return Lbf[:, rs0(p) + (i0 - p): rs0(p) + (i1 - p), :]

    inv_cur = None
    for j in range(NR):
        nsl = NR - j
        if j > 0:
            prodscr = sbt.tile([128, j, nsl, BK], BF16, name="prodscr",
                               tag="prodscr")
            for p in range(j):
                # slab product: L(i,p) * V(p,j) for i = j..8
                nc.vector.tensor_tensor(
                    prodscr[:, p, :, :], lcol(p, j, NR),
                    _bc_mid(vplane(p, j), nsl), op=ALU.mult)
            terms = list(range(j))
            while len(terms) > 1:
                nxt = []
                for q in range(0, len(terms) - 1, 2):
                    a0, a1 = terms[q], terms[q + 1]
                    nc.vector.tensor_tensor(
                        prodscr[:, a0, :, :], prodscr[:, a0, :, :],
                        prodscr[:, a1, :, :], op=ALU.add)
                    nxt.append(a0)
                if len(terms) % 2 == 1:
                    nxt.append(terms[-1])
                terms = nxt
            for i in range(j, NR):
                nc.vector.tensor_tensor(
                    vplane(j, i), vplane(j, i), prodscr[:, terms[0], i - j, :],
                    op=ALU.subtract)
        # pivot (f32, +1 for the non-border rows), logdet term, inverse
        if j < NR - 1:
            nc.vector.tensor_scalar_add(pivd[:, j, :], vplane(j, j), 1.0)
            lnd = sbt.tile([128, BK], F32, name="lnd", tag="lnd")
            nc.scalar.activation(lnd[:], pivd[:, j, :], AF.Ln)
            nc.vector.tensor_tensor(ldt[:], ldt[:], lnd[:], op=ALU.add)
            inv_cur = sbt.tile([128, BK], F32, name="invj", tag="invj")
            nc.scalar.activation(inv_cur[:], lnd[:], AF.Exp, scale=-1.0)
            # L column j (rows j+1..8)
            for i in range(j + 1, NR):
                nc.vector.tensor_tensor(
                    Lbf[:, rs0(j) + (i - j), :], vplane(j, i), inv_cur[:],
                    op=ALU.mult)
        else:
            nc.vector.tensor_copy(pivd[:, j, :], vplane(j, j))

    # ---------------- comp_logp, double logsumexp, local sum
    comp = sbt.tile([128, BK], F32, name="comp", tag="comp")
    nc.vector.tensor_tensor(comp[:], ldt[:], pivd[:, NR - 1, :], op=ALU.add)
    nc.vector.tensor_scalar(comp[:], comp[:], float(DIM * LOG2PI), -0.5,
                            op0=ALU.add, op1=ALU.mult)

    t_t = sbt.tile([128, 8, K], F32, name="t_t", tag="t_t")
    nc.vector.tensor_tensor(
        t_t[:], wall[:], comp[:].rearrange("p (c k) -> p c k", k=K),
        op=ALU.add)

    lp1 = lse_k(t_t[:], "t")
    lp = sbt.tile([128, 8], F32, name="lp", tag="lp")
    nc.vector.tensor_tensor(lp[:], lp1[:], lpw[:], op=ALU.subtract)

    lps = sbt.tile([128, 1], F32, name="lps", tag="lps")
    nc.vector.tensor_reduce(out=lps[:], in_=lp[:],
                            axis=mybir.AxisListType.X, op=ALU.add)
    ones_f = sb1.tile([128, 1], F32, name="ones_f")
    nc.vector.memset(ones_f[:], 1.0)
    pfin = ps.tile([1, 1], F32, name="pfin", tag="tr", bufs=2)
    nc.tensor.matmul(pfin[:], lhsT=lps[:], rhs=ones_f[:], start=True, stop=True)
    yt = sbt.tile([1, 1], F32, name="yt", tag="yt")
    nc.scalar.copy(yt[:], pfin[:])
    nc.sync.dma_start(out=yout[:], in_=yt[:])

    ctx.close()


# --------------------------------------------------------------- host side

_CACHE = {}


def _feature_rows():
    """ROW[t, p]: original (pre-permutation) Wout row for tile t, partition p.
    -1 = zero pad. Tiles: per kg (0..3): [mu, diag, fac r=0..7]; tile 40 = w."""
    base = K + K * DIM
    blk = DIM + DIM * RANK
    ROW = np.full((NT, 128), -1, np.int64)
    for kg in range(4):
        t0 = kg * 10
        for p in range(128):
            k = 4 * kg + p // 32
            d = p % 32
            ROW[t0 + 0, p] = K + k * DIM + d
            ROW[t0 + 1, p] = base + k * blk + d
            for r in range(RANK):
                ROW[t0 + 2 + r, p] = base + k * blk + DIM + d * RANK + r
    ROW[40, :16] = np.arange(K)
    return ROW


def _prep(inputs):
    import ml_dtypes
    bf = ml_dtypes.bfloat16
    ROW = _feature_rows()
    Wout = np.asarray(inputs["Wout"], np.float32)
    bout = np.asarray(inputs["bout"], np.float32)

    def pack_w(tiles):
        rowsl = ROW[tiles].reshape(-1)
        WP = np.zeros((len(tiles) * 128, H), np.float32)
        v = rowsl >= 0
        WP[v] = Wout[rowsl[v]]
        # [p_h, pr, i, t*128 + p_f] = WP[t*128+p_f, (2*pr+i)*128 + p_h]
        return np.ascontiguousarray(
            WP.T.reshape(2, 2, 128, len(tiles) * 128).transpose(2, 0, 1, 3))

    # bf16 tiles: mu kg (kg*10), diag kg (kg*10+1), w (40)
    bf_tiles = [kg * 10 for kg in range(4)] + \
               [kg * 10 + 1 for kg in range(4)] + [40]
    woutt = pack_w(bf_tiles).astype(bf)
    # fp8 tiles: fac (kg*10+2+r), kg-major then r
    f8_tiles = [kg * 10 + 2 + r for kg in range(4) for r in range(RANK)]
    wout8 = pack_w(f8_tiles).astype(ml_dtypes.float8_e4m3)

    rows = ROW.reshape(-1)
    valid = rows >= 0
    bias_full = np.zeros((NT, 128), np.float32)
    bias_full[valid.reshape(NT, 128)] = bout[rows[valid]]
    biasft = np.zeros((128, 45), np.float32)
    for kg in range(4):
        t0 = kg * 10
        biasft[:, kg] = bias_full[t0 + 0]
        biasft[:, 4 + kg] = bias_full[t0 + 1]
        biasft[:, 8 + kg] = -0.5 * bias_full[t0 + 1]
        for r in range(RANK):
            biasft[:, 12 + kg * 8 + r] = bias_full[t0 + 2 + r]
    biasft[:, 44] = bias_full[40]

    w0t = np.ascontiguousarray(np.asarray(inputs["W0"], np.float32).T).astype(bf)
    wht = np.ascontiguousarray(
        np.transpose(np.asarray(inputs["Wh"], np.float32), (0, 2, 1))).astype(bf)

    def v128(v):
        return np.ascontiguousarray(np.asarray(v, np.float32).reshape(4, 128).T)

    vec_list = [inputs["b0"], inputs["g0"], inputs["be0"]]
    for li in range(NL - 1):
        vec_list += [inputs["bh"][li], inputs["gh"][li], inputs["beh"][li]]
    vecs = np.stack([v128(v) for v in vec_list], axis=-1).astype(np.float32)

    pat8 = np.zeros((128, 8, 32), np.float32)
    for p in range(8):
        for i in range(4):
            pat8[32 * i:32 * (i + 1), p, 4 * p + i] = 1.0
    pat8 = pat8.astype(bf)
    ident = np.eye(128, dtype=np.float32).astype(bf)
    eye16 = np.eye(16, dtype=np.float32)

    data = np.asarray(inputs["data"], np.float32)
    context = np.asarray(inputs["context"], np.float32)
    in_maps = []
    for c in range(N_CORES):
        sl = slice(c * BL, (c + 1) * BL)
        in_maps.append({
            "ctxT": np.ascontiguousarray(context[sl].T).astype(bf),
            "dataT": np.ascontiguousarray(data[sl].T),
            "w0t": w0t, "wht": wht, "woutt": woutt, "wout8": wout8,
            "biasft": biasft,
            "vecs": vecs, "pat8": pat8, "ident": ident, "eye16": eye16,
        })
    return in_maps


def kernel(**inputs):
    from concourse.bass_utils import run_bass_kernel_spmd

    if "nc" not in _CACHE:
        _CACHE["nc"] = build_program()
    nc = _CACHE["nc"]
    in_maps = _prep(inputs)
    res = run_bass_kernel_spmd(nc, in_maps, core_ids=list(range(N_CORES)))
    total = sum(float(res.results[c]["yout"][0, 0]) for c in range(N_CORES))
    return np.float32(-total / B)


# revision 35
# speedup vs baseline: 1.0041x; 1.0041x over previous
"""LowRankMixtureDensityNetwork loss on 8 Trainium2 NeuronCores.

Data-parallel over the batch (1024 rows/core), MLP weights replicated.
BatchNorm (training mode) statistics are allreduced across cores per layer.

Tail strategy (v3): the output layer and mixture tail run FEATURE-on-
partition. Output feature tiles are [(k4, d32), b]; per-partition biases
are applied on ACT during psum evacuation. The bordered low-rank rows
A_r = (fac_r + b) * exp(-diag/2) and u = -(data - mu) * exp(-diag/2) are
built elementwise; Gram pair products run as full-batch diagonal slabs on
DVE/GpSimd, and the d-reduction is a TensorE matmul against one of eight
block-diagonal ones stationaries, scattering each pair's [4k, b] fold into
32-partition groups of shared PSUM tiles (tile_position column quadrants;
groups issue strictly sequentially). Small TensorE transposes flip the
capacitance entries back to batch-on-partition for the bordered 9x9 LDL,
whose slots are diagonal-major compact with the +I folded into the pivot
extraction (column accesses are per-plane).
"""
import contextlib

import numpy as np

import concourse.bass as bass
import concourse.tile as tile
from concourse import mybir
import bass_rust

F32 = mybir.dt.float32
FP8 = mybir.dt.float8e4
BF16 = mybir.dt.bfloat16
AF = mybir.ActivationFunctionType
ALU = mybir.AluOpType

# problem constants
DIM, K, RANK = 32, 16, 8
CTX, H, NL, B = 128, 512, 4, 8192
OUT = K + DIM * K + (DIM + DIM * RANK) * K          # 5136
N_CORES = 8
BL = B // N_CORES                                    # 1024 rows per core
NR = RANK + 1                                        # 9 (bordered system)
NT = 41                                              # feature tiles (40 + w)
LOG2PI = float(np.log(2.0 * np.pi))

# diagonal-major compact slot base: pair (a, b), a<=b -> DIAGBASE[b-a] + a
DIAGBASE = [0, 9, 17, 24, 30, 35, 39, 42, 44]        # 45 pair slots
NSLOT = 46                                           # +1 for ldraw
NU_F1 = 32

# fold positions issue in diagonal-major order (0..44), then ldraw (45):
# psum groups are strictly sequential. Position 45 (ldraw) gets its own
# 32-aligned group (F2 rows 64..67) so its ACT evac is quadrant-aligned.
_GFIRST = {0: 0, 1: 8, 2: 16, 3: 24, 4: 32, 5: 40, 6: 45}
_GLAST = {0: 7, 1: 15, 2: 23, 3: 31, 4: 39, 5: 44, 6: 45}

# engine split knobs
GRAM_GPS_O = (4, 5, 6, 7, 8)   # Gram diagonals whose products run on GpSimd
ABUILD_GPS_R = (5, 6, 7)       # A-build rows multiplied on GpSimd

# ------------------------------------------------------------- walrus quirks

_ctr = [0]


def _split_multi_waits(nc, max_waits=1):
    """walrus in this container rejects >1 sync wait per instruction; hoist
    excess waits onto same-engine NOPs placed just before the instruction."""
    n_split = 0
    for f in nc.m.functions:
        for bb in f.blocks:
            insts = bb.instructions
            out = []
            changed = False
            for inst in insts:
                si = inst.sync_info
                waits = list(si.on_wait) if si is not None else []
                if len(waits) > max_waits:
                    for w in waits[:-max_waits]:
                        _ctr[0] += 1
                        nop = mybir.InstNoOp(
                            name=f"WSPLIT-{_ctr[0]}",
                            engine=inst.engine,
                            ins=[],
                            outs=[],
                            sync_info=mybir.SyncInfo(on_wait=[w], on_update=[]),
                        )
                        out.append(nop)
                    inst.sync_info = mybir.SyncInfo(
                        on_wait=waits[-max_waits:], on_update=list(si.on_update)
                    )
                    changed = True
                    n_split += 1
                out.append(inst)
            if changed:
                bb.instructions = out
    return n_split


def _patched_drain_and_barrier(self, tick_clock, wait_clock):
    nc = self.nc
    probe = nc.sync.nop()
    wait_clock.add_sem_waits(
        probe.ins, bass_rust.ScopedClock({None: tick_clock.global_clock})
    )
    si = probe.ins.sync_info
    waits = list(si.on_wait) if si is not None else []
    if len(waits) > 1:
        probe.ins.sync_info = mybir.SyncInfo(on_wait=waits[:1], on_update=[])
        for w in waits[1:]:
            extra = nc.sync.nop()
            extra.ins.sync_info = mybir.SyncInfo(on_wait=[w], on_update=[])
    nc.sync.drain()

    nc.all_engine_barrier()
    assert self.sems is not None
    popped = nc._tile_sem_poison_stack.pop()
    assert popped is self._sem_poison
    nc.clear_and_free_semaphores(list(self.sems.allocated().values()))
    nc.all_engine_barrier()


tile.TileContext._drain_and_barrier = _patched_drain_and_barrier


def _bc_mid(ap, n):
    """[P, inner] AP -> [P, n, inner] with a stride-0 middle dim"""
    return bass.AP(tensor=ap.tensor, offset=ap.offset,
                   ap=[ap.ap[0], [0, n], ap.ap[-1]])


def _bc_inner(ap, k):
    """[P, n] AP -> [P, n, k] with a stride-0 inner dim"""
    return bass.AP(tensor=ap.tensor, offset=ap.offset,
                   ap=[ap.ap[0], ap.ap[-1], [0, k]])


# ----------------------------------------------------------------- program


def build_program(split=True):
    nc = bass.Bass("TRN2", num_devices=N_CORES)

    ctxT = nc.dram_tensor("ctxT", [CTX, BL], BF16, kind="ExternalInput")
    dataT = nc.dram_tensor("dataT", [DIM, BL], F32, kind="ExternalInput")
    w0t = nc.dram_tensor("w0t", [CTX, H], BF16, kind="ExternalInput")
    wht = nc.dram_tensor("wht", [NL - 1, H, H], BF16, kind="ExternalInput")
    woutt = nc.dram_tensor("woutt", [128, 2, 2, 9 * 128], BF16,
                           kind="ExternalInput")
    wout8 = nc.dram_tensor("wout8", [128, 2, 2, 32 * 128],
                           mybir.dt.float8e4, kind="ExternalInput")
    biasft = nc.dram_tensor("biasft", [128, 45], F32, kind="ExternalInput")
    # per-feature vectors packed [128, 4hc, 12]:
    #   0:b0 1:g0 2:be0, then per hidden l (0..2): 3+3l:bh, 4+3l:gh, 5+3l:beh
    vecs = nc.dram_tensor("vecs", [128, 4, 12], F32, kind="ExternalInput")
    pat8 = nc.dram_tensor("pat8", [128, 8, 32], BF16, kind="ExternalInput")
    ident = nc.dram_tensor("ident", [128, 128], BF16, kind="ExternalInput")
    eye16 = nc.dram_tensor("eye16", [16, 16], F32, kind="ExternalInput")
    yout = nc.dram_tensor("yout", [1, 1], F32, kind="ExternalOutput")

    with tile.TileContext(nc) as tc:
        _body(nc, tc, ctxT, dataT, w0t, wht, woutt, wout8, biasft, vecs,
              pat8, ident, eye16, yout)
    if split:
        _split_multi_waits(nc)
    return nc


def _mlp(nc, tc, sb1, dram, ctxT, w0t, wht, vecs):
    """feature-on-partition MLP with cross-core BN; returns u3p (bf16)."""
    ctx = contextlib.ExitStack()
    sbm = ctx.enter_context(tc.tile_pool(name="mlpwork", bufs=2))
    sbu = ctx.enter_context(tc.tile_pool(name="uacts", bufs=2))
    sbe = ctx.enter_context(tc.tile_pool(name="elu", bufs=3))
    ps = ctx.enter_context(tc.tile_pool(name="psm", bufs=1, space="PSUM"))

    t_ctx = sbm.tile([128, BL], BF16, name="t_ctx", tag="t_ctx", bufs=1)
    nc.sync.dma_start(out=t_ctx[:], in_=ctxT[:])
    t_w0 = sbm.tile([128, H], BF16, name="t_w0", tag="t_w0", bufs=1)
    nc.sync.dma_start(out=t_w0[:], in_=w0t[:])
    t_wh = sbm.tile([128, NL - 1, 2, 2, H], BF16, name="t_wh", tag="t_wh",
                    bufs=1)
    nc.sync.dma_start(
        out=t_wh[:], in_=wht.rearrange("l (r i p) m -> p l r i m", r=2, i=2))
    t_vec = sbm.tile([128, 4, 12], F32, name="t_vec", tag="t_vec", bufs=1)
    nc.sync.dma_start(out=t_vec[:], in_=vecs[:])
    eps_t = sbm.tile([128, 1], F32, name="eps_t", tag="eps_t", bufs=1)
    nc.vector.memset(eps_t[:], 1e-5)

    # collective warmup (absorb first-collective latency)
    cwu_in = dram.tile([128, 1], F32, name="cwu_in")
    cwu_out = dram.tile([128, 1], F32, name="cwu_out")
    t_junk = sbm.tile([128, 1], F32, name="t_junk", tag="t_junk", bufs=1)
    nc.vector.memset(t_junk[:], 0.0)
    nc.sync.dma_start(out=cwu_in[:], in_=t_junk[:])
    nc.gpsimd.collective_compute(
        "AllReduce", ALU.add, replica_groups=[list(range(N_CORES))],
        ins=[cwu_in[:].opt()], outs=[cwu_out[:].opt()],
    )
    nc.gpsimd.collective_compute(
        "AllReduce", ALU.add, replica_groups=[list(range(N_CORES))],
        ins=[cwu_out[:].opt()], outs=[cwu_in[:].opt()],
    )
    t_junk2 = sbm.tile([128, 1], F32, name="t_junk2", tag="t_junk2", bufs=1)
    nc.gpsimd.dma_start(out=t_junk2[:], in_=cwu_in[:])

    u_prev = None
    u3p = None
    wfold = None
    beff = None

    for layer in range(NL):
        u_cur = sbu.tile([128, 4, BL], BF16, name=f"u{layer}", tag="u")
        nkc = 1 if layer == 0 else 4
        for hc in range(4):
            if layer == 0:
                bcol = t_vec[:, hc, 0:1]
            else:
                bcol = beff[:, hc:hc + 1]
            for bcc in range(2):
                bs = bcc * 512
                psum = ps.tile([128, 512], F32, name="zp", tag="z", bufs=3)
                if layer == 0:
                    nc.tensor.matmul(psum[:],
                                     lhsT=t_w0[:, hc * 128:(hc + 1) * 128],
                                     rhs=t_ctx[:, bs:bs + 512],
                                     start=True, stop=True)
                else:
                    for kc in range(4):
                        nc.tensor.matmul(
                            psum[:],
                            lhsT=wfold[:, kc // 2, kc % 2,
                                       hc * 128:(hc + 1) * 128],
                            rhs=u_prev[:, kc, bs:bs + 512],
                            start=(kc == 0), stop=(kc == 3))
                # ELU: u = min(exp(z+b) - 1, relu(z+b))
                e_t = sbe.tile([128, 512], F32, name="elu_e", tag="elu_e")
                nc.scalar.activation(e_t[:], psum[:], AF.Exp, bias=bcol)
                r_t = sbe.tile([128, 512], BF16, name="elu_r", tag="elu_r")
                nc.scalar.activation(r_t[:], psum[:], AF.Relu, bias=bcol)
                nc.vector.scalar_tensor_tensor(
                    u_cur[:, hc, bs:bs + 512], e_t[:], -1.0, r_t[:],
                    op0=ALU.add, op1=ALU.min)

        # ---- batch-norm stats (local) -> allreduce -> affine params
        stats = sbm.tile([128, 4, 2, 6], F32, name="bns", tag="bns")
        for hc in range(4):
            for half in range(2):
                nc.vector.bn_stats(
                    out=stats[:, hc, half, :],
                    in_=u_cur[:, hc, half * 512:(half + 1) * 512])
        mv = sbm.tile([128, 4, 2], F32, name="bnmv", tag="bnmv")
        for hc in range(4):
            nc.vector.bn_aggr(out=mv[:, hc, :], in_=stats[:, hc, :, :])
        pack = sbm.tile([128, 8], F32, name="bnp", tag="bnp")
        mm = mv[:, :, 0:1].rearrange("p h one -> p (h one)")
        vv = mv[:, :, 1:2].rearrange("p h one -> p (h one)")
        nc.vector.tensor_scalar_mul(pack[:, 0:4], mm, float(BL))
        msq = sbm.tile([128, 4], F32, name="bmsq", tag="bmsq")
        nc.vector.tensor_tensor(msq[:], mm, mm, op=ALU.mult)
        s2s = sbm.tile([128, 4], F32, name="bs2", tag="bs2")
        nc.vector.tensor_tensor(s2s[:], vv, msq[:], op=ALU.add)
        nc.vector.tensor_scalar_mul(pack[:, 4:8], s2s[:], float(BL))

        ar_in = dram.tile([128, 8], F32, name=f"arin{layer}")
        ar_out = dram.tile([128, 8], F32, name=f"arout{layer}")
        nc.sync.dma_start(out=ar_in[:], in_=pack[:])
        nc.gpsimd.collective_compute(
            "AllReduce", ALU.add, replica_groups=[list(range(N_CORES))],
            ins=[ar_in[:].opt()], outs=[ar_out[:].opt()],
        )
        warm = ps.tile([128, 512], F32, name="warm", tag="warm", bufs=1)
        for _ in range(28):
            nc.tensor.matmul(warm[:], lhsT=t_w0[:, 0:128],
                             rhs=t_ctx[:, 0:512], start=True, stop=True)
        red = sbm.tile([128, 8], F32, name="bnr", tag="bnr")
        nc.gpsimd.dma_start(out=red[:], in_=ar_out[:])

        iv = 0 if layer == 0 else 3 * (layer - 1) + 3
        g_col = t_vec[:, :, iv + 1]
        be_col = t_vec[:, :, iv + 2]
        m_t = sbm.tile([128, 4], F32, name="bnm", tag="bnm")
        nc.vector.tensor_scalar_mul(m_t[:], red[:, 0:4], 1.0 / B)
        msq2 = sbm.tile([128, 4], F32, name="bnm2", tag="bnm2")
        nc.vector.tensor_tensor(msq2[:], m_t[:], m_t[:], op=ALU.mult)
        var_t = sbm.tile([128, 4], F32, name="bnv", tag="bnv")
        nc.vector.scalar_tensor_tensor(
            var_t[:], red[:, 4:8], 1.0 / B, msq2[:],
            op0=ALU.mult, op1=ALU.subtract)
        # a = g * rsqrt(var+eps) = g * exp(-0.5*ln(var+eps))
        lnv = sbm.tile([128, 4], F32, name="bnl", tag="bnl")
        nc.scalar.activation(lnv[:], var_t[:], AF.Ln, bias=eps_t[:])
        rsq = sbm.tile([128, 4], F32, name="bnq", tag="bnq")
        nc.scalar.activation(rsq[:], lnv[:], AF.Exp, scale=-0.5)
        a_t = sbm.tile([128, 4], F32, name="bna", tag="bna")
        nc.vector.tensor_tensor(a_t[:], g_col, rsq[:], op=ALU.mult)
        ma = sbm.tile([128, 4], F32, name="bnma", tag="bnma")
        nc.vector.tensor_tensor(ma[:], m_t[:], a_t[:], op=ALU.mult)
        c_t = sbm.tile([128, 4], F32, name="bnc", tag="bnc")
        nc.vector.tensor_tensor(c_t[:], be_col, ma[:], op=ALU.subtract)

        if layer < NL - 1:
            # fold affine into next layer: W' = WhT * a (per contraction row)
            wfold = sbm.tile([128, 2, 2, H], BF16, name="wf", tag="wf")
            for kc in range(4):
                nc.vector.tensor_scalar_mul(
                    wfold[:, kc // 2, kc % 2, :],
                    t_wh[:, layer, kc // 2, kc % 2, :], a_t[:, kc:kc + 1])
            # bias: z_{l+1} = W'u + (Wh[layer] @ c + b_{l+1})
            c_bf = sbm.tile([128, 4], BF16, name="cbf", tag="cbf")
            nc.vector.tensor_copy(c_bf[:], c_t[:])
            beff = sbm.tile([128, 4], F32, name="beff", tag="beff")
            b_next = t_vec[:, :, 3 * layer + 3]
            for mc in range(4):
                pb = ps.tile([128, 1], F32, name="pbias", tag="pbias", bufs=1)
                for kc in range(4):
                    nc.tensor.matmul(
                        pb[:],
                        lhsT=t_wh[:, layer, kc // 2, kc % 2,
                                  mc * 128:(mc + 1) * 128],
                        rhs=c_bf[:, kc:kc + 1],
                        start=(kc == 0), stop=(kc == 3))
                nc.scalar.activation(
                    beff[:, mc:mc + 1], pb[:], AF.Identity,
                    bias=b_next[:, mc:mc + 1])
            u_prev = u_cur
        else:
            # BN3 applied directly on u (Wout stays raw)
            u3p = sb1.tile([128, 4, BL], BF16, name="u3p")
            u3p8 = sb1.tile([128, 4, BL], FP8, name="u3p8")
            for hc in range(4):
                nc.scalar.activation(
                    u3p[:, hc, :], u_cur[:, hc, :], AF.Identity,
                    bias=c_t[:, hc:hc + 1], scale=a_t[:, hc:hc + 1])
                nc.scalar.activation(
                    u3p8[:, hc, :], u_cur[:, hc, :], AF.Identity,
                    bias=c_t[:, hc:hc + 1], scale=a_t[:, hc:hc + 1])

    ctx.close()
    return u3p, u3p8


def _body(nc, tc, ctxT, dataT, w0t, wht, woutt, wout8, biasft, vecs,
          pat8, ident, eye16, yout):
    ctx = contextlib.ExitStack()
    sb1 = ctx.enter_context(tc.tile_pool(name="persist", bufs=1))
    dram = ctx.enter_context(tc.tile_pool(name="dram", bufs=1, space="DRAM"))

    t_wo = sb1.tile([128, 2, 2, 9 * 128], BF16, name="t_wo")
    nc.sync.dma_start(out=t_wo[:], in_=woutt[:])
    t_wo8 = sb1.tile([128, 2, 2, 32 * 128], FP8, name="t_wo8")
    nc.sync.dma_start(out=t_wo8[:], in_=wout8[:])
    t_bias = sb1.tile([128, 45], F32, name="t_bias")
    nc.sync.dma_start(out=t_bias[:], in_=biasft[:])
    t_pat = sb1.tile([128, 8, 32], BF16, name="t_pat")
    nc.sync.dma_start(out=t_pat[:], in_=pat8[:])
    t_id = sb1.tile([128, 128], BF16, name="t_id")
    nc.sync.dma_start(out=t_id[:], in_=ident[:])
    t_eye16 = sb1.tile([16, 16], F32, name="t_eye16")
    nc.sync.dma_start(out=t_eye16[:], in_=eye16[:])
    dataRep = sb1.tile([128, BL], F32, name="dataRep")
    for g in range(4):
        nc.sync.dma_start(out=dataRep[32 * g:32 * (g + 1), :], in_=dataT[:])

    u3p, u3p8 = _mlp(nc, tc, sb1, dram, ctxT, w0t, wht, vecs)
    ps = ctx.enter_context(tc.tile_pool(name="ps", bufs=1, space="PSUM"))

    # persistent tail state
    capG = sb1.tile([128, NSLOT, 128], BF16, name="capG")   # slot, (c8,k16)
    ldall = sb1.tile([128, 8, K], F32, name="ldall")
    wall = sb1.tile([128, 8, K], F32, name="wall")

    sbt = ctx.enter_context(tc.tile_pool(name="tail", bufs=2))

    # bias column helpers (feature-on-partition, per kg)
    def b_mu(kg):
        return t_bias[:, kg:kg + 1]

    def b_diag(kg):
        return t_bias[:, 4 + kg:5 + kg]

    def b_dhalf(kg):
        return t_bias[:, 8 + kg:9 + kg]

    def b_fac(kg, r):
        c = 12 + kg * 8 + r
        return t_bias[:, c:c + 1]

    b_w = t_bias[0:16, 44:45]

    state = {}

    def stage_A(kg, half):
        """output GEMMs + A-row builds for (kg, half) (PE/ACT/DVE/GPS)."""
        if half == 0:
            state[kg] = {
                "At": sbt.tile([128, NR, BL], BF16, name="At", tag="At",
                               bufs=2),
                "ldb": sbt.tile([128, BL], BF16, name="ldb", tag="ldb",
                                bufs=2),
            }
        At = state[kg]["At"]
        ldb_bf = state[kg]["ldb"]
        bs = half * 512

        def gemm(t):
            pg = ps.tile([128, 512], F32, name="pg", tag="gemm", bufs=2)
            for kc in range(4):
                nc.tensor.matmul(
                    pg[:], lhsT=t_wo[:, kc // 2, kc % 2,
                                     t * 128:(t + 1) * 128],
                    rhs=u3p[:, kc, bs:bs + 512],
                    start=(kc == 0), stop=(kc == 3))
            return pg

        # diag tile: s = exp(-(raw+b)/2), ldb = raw+b
        pg = gemm(4 + kg)
        s_bf = sbt.tile([128, 512], BF16, name="s_bf", tag="s_bf")
        nc.scalar.activation(s_bf[:], pg[:], AF.Exp,
                             bias=b_dhalf(kg), scale=-0.5)
        nc.scalar.activation(ldb_bf[:, bs:bs + 512], pg[:], AF.Identity,
                             bias=b_diag(kg))

        # mu tile: diffn = (mu+b) - data (negated diff; sign cancels)
        pg = gemm(kg)
        diffn = sbt.tile([128, 512], BF16, name="diffn", tag="diffn")
        nc.vector.scalar_tensor_tensor(
            diffn[:], pg[:], b_mu(kg), dataRep[:, bs:bs + 512],
            op0=ALU.add, op1=ALU.subtract)
        nc.vector.tensor_tensor(At[:, RANK, bs:bs + 512], diffn[:],
                                s_bf[:], op=ALU.mult)

        # A rows: fused (fac+b)*s from psum on DVE; some rows via
        # ACT evac + GpSimd multiply to spread engine load.
        # fac GEMMs run in fp8 DoubleRow (256-deep contraction).
        def gemm8(t):
            pg = ps.tile([128, 512], F32, name="pg", tag="gemm", bufs=2)
            for pr in range(2):
                nc.tensor.matmul(
                    pg[:], lhsT=t_wo8[:, pr, :, t * 128:(t + 1) * 128],
                    rhs=u3p8[:, 2 * pr:2 * pr + 2, bs:bs + 512],
                    start=(pr == 0), stop=(pr == 1),
                    perf_mode=mybir.MatmulPerfMode.DoubleRow)
            return pg

        for r in range(RANK):
            pg = gemm8(kg * 8 + r)
            if r in ABUILD_GPS_R:
                fbf = sbt.tile([128, 512], BF16, name="fbf", tag="fbf",
                               bufs=4)
                nc.scalar.activation(fbf[:], pg[:], AF.Identity,
                                     bias=b_fac(kg, r))
                nc.gpsimd.tensor_tensor(At[:, r, bs:bs + 512], fbf[:],
                                        s_bf[:], op=ALU.mult)
            else:
                nc.vector.scalar_tensor_tensor(
                    At[:, r, bs:bs + 512], pg[:], b_fac(kg, r), s_bf[:],
                    op0=ALU.add, op1=ALU.mult)

    def stage_BC(kg, h):
        """Gram products (DVE/GPS) + fold matmuls (PE) for (kg, half)."""
        At = state[kg]["At"]
        ldb_bf = state[kg]["ldb"]
        f1 = ps.tile([128, 512], F32, name=f"f1h{h}", tag=f"F1{h}", bufs=1)
        f2 = ps.tile([96, 512], F32, name=f"f2h{h}", tag=f"F2{h}", bufs=1)
        state[kg][f"f1h{h}"], state[kg][f"f2h{h}"] = f1, f2
        hs = 512 * h

        def fold(pos, mov):
            if pos == 45:
                g, p = 6, 0
                out, tp = f2[64:96, :], (0, 64)
            else:
                g, p = divmod(pos, 8)
                if pos < NU_F1:
                    out, tp = f1[32 * g:32 * g + 32, :], (0, 32 * g)
                else:
                    out = f2[32 * (g - 4):32 * (g - 4) + 32, :]
                    tp = (0, 32 * (g - 4))
            nc.tensor.matmul(out, lhsT=t_pat[:, p, :], rhs=mov,
                             start=(_GFIRST[g] == pos),
                             stop=(_GLAST[g] == pos), tile_position=tp)

        pos = 0
        for o in range(NR):
            n = NR - o
            P = sbt.tile([128, NR, 512], BF16, name="P", tag="P", bufs=4)
            eng = nc.gpsimd if o in GRAM_GPS_O else nc.vector
            if o < 2:  # split big slabs: let the first folds start earlier
                nsp = n // 2
                eng.tensor_tensor(P[:, 0:nsp, :], At[:, 0:nsp, hs:hs + 512],
                                  At[:, o:o + nsp, hs:hs + 512], op=ALU.mult)
                eng.tensor_tensor(P[:, nsp:n, :], At[:, nsp:n, hs:hs + 512],
                                  At[:, o + nsp:NR, hs:hs + 512], op=ALU.mult)
            else:
                eng.tensor_tensor(P[:, 0:n, :], At[:, 0:n, hs:hs + 512],
                                  At[:, o:NR, hs:hs + 512], op=ALU.mult)
            for a in range(n):
                fold(pos, P[:, a, :])
                pos += 1
        fold(45, ldb_bf[:, hs:hs + 512])

    def stage_D(kg):
        """fold psum evac + transposes + capG scatter for kg (ACT/PE)."""
        cf1 = sbt.tile([128, BL], BF16, name="cf1", tag="cf1")
        cf2 = sbt.tile([52, BL], BF16, name="cf2", tag="cf2")
        ldb32 = sbt.tile([4, BL], F32, name="ldb32", tag="ldb32")
        for h in range(2):
            hs = 512 * h
            f1, f2 = state[kg][f"f1h{h}"], state[kg][f"f2h{h}"]
            nc.scalar.copy(cf1[:, hs:hs + 512], f1[:])
            nc.scalar.copy(cf2[:, hs:hs + 512], f2[0:52, :])
            nc.scalar.copy(ldb32[:, hs:hs + 512], f2[64:68, :])
        for c in range(8):
            ioff = c * K + kg * 4
            pt1 = ps.tile([128, 128], BF16, name="pt1", tag="tr", bufs=2)
            nc.tensor.transpose(pt1[:], cf1[:, c * 128:(c + 1) * 128],
                                t_id[:])
            nc.scalar.copy(capG[:, 0:32, ioff:ioff + 4],
                           pt1[:].rearrange("p (u i) -> p u i", i=4))
            pt2f = ps.tile([128, 128], BF16, name="pt2", tag="tr", bufs=2)
            pt2 = pt2f[:, 0:52]
            nc.tensor.transpose(pt2, cf2[:, c * 128:(c + 1) * 128],
                                t_id[0:52, 0:52])
            nc.scalar.copy(capG[:, 32:45, ioff:ioff + 4],
                           pt2.rearrange("p (u i) -> p u i", i=4))
            pt3f = ps.tile([128, 16], F32, name="pt3", tag="tr", bufs=2)
            pt3 = pt3f[:, 0:4]
            nc.tensor.transpose(pt3, ldb32[:, c * 128:(c + 1) * 128],
                                t_eye16[0:4, 0:4])
            nc.scalar.copy(ldall[:, c, kg * 4:(kg + 1) * 4], pt3)
        del state[kg]

    # software pipeline at half-batch granularity: the PE streams the next
    # half's GEMMs while DVE/GPS run the current half's Gram products.
    stage_A(0, 0)
    stage_A(0, 1)
    stage_BC(0, 0)
    stage_A(1, 0)
    stage_BC(0, 1)
    stage_A(1, 1)
    stage_D(0)
    stage_BC(1, 0)
    stage_A(2, 0)
    stage_BC(1, 1)
    stage_A(2, 1)
    stage_D(1)
    stage_BC(2, 0)
    stage_A(3, 0)
    stage_BC(2, 1)
    stage_A(3, 1)
    stage_D(2)
    stage_BC(3, 0)
    stage_BC(3, 1)
    stage_D(3)

    # ---- w tile: logits to batch layout
    wsb = sbt.tile([16, BL], F32, name="wsb", tag="wsb", bufs=1)
    for half in range(2):
        bs = half * 512
        pgw = ps.tile([128, 512], F32, name="pgw", tag="gemm", bufs=2)
        for kc in range(4):
            nc.tensor.matmul(pgw[:],
                             lhsT=t_wo[:, kc // 2, kc % 2, 8 * 128:9 * 128],
                             rhs=u3p[:, kc, bs:bs + 512],
                             start=(kc == 0), stop=(kc == 3))
        nc.scalar.activation(wsb[:, bs:bs + 512], pgw[0:16, :], AF.Identity,
                             bias=b_w)
    for c in range(8):
        ptw = ps.tile([128, 16], F32, name="ptw", tag="tr", bufs=2)
        nc.tensor.transpose(ptw[:], wsb[:, c * 128:(c + 1) * 128],
                            t_eye16[:])
        nc.scalar.copy(wall[:, c, :], ptw[:])

    # ---------------- bordered slab LDL over [128, 128] planes
    # V lives in capG diagonal-major compact slots (entry (a,b) at
    # DIAGBASE[b-a]+a); +I folded into the pivot extraction. Column access
    # is per-plane (non-constant slot stride across diagonals).
    # L is compact r-major: (i,p) at rs0(p)+(i-p)
    def rs0(r):
        return r * NR - r * (r - 1) // 2

    BK = 128
    Lbf = sb1.tile([128, 45, BK], BF16, name="Lbf")
    pivd = sb1.tile([128, NR, BK], F32, name="pivd")
    ldt = sb1.tile([128, BK], F32, name="ldt")
    nc.vector.tensor_copy(ldt[:], ldall[:].rearrange("p c k -> p (c k)"))

    def vplane(a, b):
        """cap entry (a, b) with a<=b -> [128, BK]"""
        return capG[:, DIAGBASE[b - a] + a, :]

    def lcol(p, i0, i1):
        return Lbf[:, rs0(p) + (i0 - p): rs0(p) + (i1 - p), :]

    inv_cur = None
    for j in range(NR):
        nsl = NR - j
        if j > 0:
            prodscr = sbt.tile([128, j, nsl, BK], BF16, name="prodscr",
                               tag="prodscr")
            for p in range(j):
                # slab product: L(i,p) * V(p,j) for i = j..8
                nc.vector.tensor_tensor(
                    prodscr[:, p, :, :], lcol(p, j, NR),
                    _bc_mid(vplane(p, j), nsl), op=ALU.mult)
            terms = list(range(j))
            while len(terms) > 1:
                nxt = []
                for q in range(0, len(terms) - 1, 2):
                    a0, a1 = terms[q], terms[q + 1]
                    nc.vector.tensor_tensor(
                        prodscr[:, a0, :, :], prodscr[:, a0, :, :],
                        prodscr[:, a1, :, :], op=ALU.add)
                    nxt.append(a0)
                if len(terms) % 2 == 1:
                    nxt.append(terms[-1])
                terms = nxt
            for i in range(j, NR):
                nc.vector.tensor_tensor(
                    vplane(j, i), vplane(j, i), prodscr[:, terms[0], i - j, :],
                    op=ALU.subtract)
        # pivot (f32, +1 for the non-border rows), logdet term, inverse
        if j < NR - 1:
            nc.vector.tensor_scalar_add(pivd[:, j, :], vplane(j, j), 1.0)
            lnd = sbt.tile([128, BK], F32, name="lnd", tag="lnd")
            nc.scalar.activation(lnd[:], pivd[:, j, :], AF.Ln)
            nc.vector.tensor_tensor(ldt[:], ldt[:], lnd[:], op=ALU.add)
            inv_cur = sbt.tile([128, BK], F32, name="invj", tag="invj")
            nc.scalar.activation(inv_cur[:], lnd[:], AF.Exp, scale=-1.0)
            # L column j (rows j+1..8)
            for i in range(j + 1, NR):
                nc.vector.tensor_tensor(
                    Lbf[:, rs0(j) + (i - j), :], vplane(j, i), inv_cur[:],
                    op=ALU.mult)
        else:
            nc.vector.tensor_copy(pivd[:, j, :], vplane(j, j))

    # ---------------- comp_logp, double logsumexp, local sum
    comp = sbt.tile([128, BK], F32, name="comp", tag="comp")
    nc.vector.tensor_tensor(comp[:], ldt[:], pivd[:, NR - 1, :], op=ALU.add)
    nc.vector.tensor_scalar(comp[:], comp[:], float(DIM * LOG2PI), -0.5,
                            op0=ALU.add, op1=ALU.mult)

    t_t = sbt.tile([128, 8, K], F32, name="t_t", tag="t_t")
    nc.vector.tensor_tensor(
        t_t[:], wall[:], comp[:].rearrange("p (c k) -> p c k", k=K),
        op=ALU.add)

    def lse_k(src3d, nm):
        mx = sbt.tile([128, 8], F32, name=f"mx{nm}", tag=f"mx{nm}")
        nc.vector.tensor_reduce(out=mx[:], in_=src3d,
                                axis=mybir.AxisListType.X, op=ALU.max)
        zs = sbt.tile([128, 8, K], F32, name=f"zs{nm}", tag=f"zs{nm}")
        nc.vector.tensor_tensor(zs[:], src3d, _bc_inner(mx[:], K),
                                op=ALU.subtract)
        ez = sbt.tile([128, 8, K], F32, name=f"ez{nm}", tag=f"ez{nm}")
        nc.scalar.activation(ez[:], zs[:], AF.Exp)
        sez = sbt.tile([128, 8], F32, name=f"se{nm}", tag=f"se{nm}")
        nc.vector.tensor_reduce(out=sez[:], in_=ez[:],
                                axis=mybir.AxisListType.X, op=ALU.add)
        ls = sbt.tile([128, 8], F32, name=f"ls{nm}", tag=f"ls{nm}")
        nc.scalar.activation(ls[:], sez[:], AF.Ln)
        out = sbt.tile([128, 8], F32, name=f"lo{nm}", tag=f"lo{nm}")
        nc.vector.tensor_tensor(out[:], mx[:], ls[:], op=ALU.add)
        return out

    lp1 = lse_k(t_t[:], "t")
    lpw = lse_k(wall[:], "w")
    lp = sbt.tile([128, 8], F32, name="lp", tag="lp")
    nc.vector.tensor_tensor(lp[:], lp1[:], lpw[:], op=ALU.subtract)

    lps = sbt.tile([128, 1], F32, name="lps", tag="lps")
    nc.vector.tensor_reduce(out=lps[:], in_=lp[:],
                            axis=mybir.AxisListType.X, op=ALU.add)
    ones_f = sb1.tile([128, 1], F32, name="ones_f")
    nc.vector.memset(ones_f[:], 1.0)
    pfin = ps.tile([1, 1], F32, name="pfin", tag="tr", bufs=2)
    nc.tensor.matmul(pfin[:], lhsT=lps[:], rhs=ones_f[:], start=True, stop=True)
    yt = sbt.tile([1, 1], F32, name="yt", tag="yt")
    nc.scalar.copy(yt[:], pfin[:])
    nc.sync.dma_start(out=yout[:], in_=yt[:])

    ctx.close()


# --------------------------------------------------------------- host side

_CACHE = {}


def _feature_rows():
    """ROW[t, p]: original (pre-permutation) Wout row for tile t, partition p.
    -1 = zero pad. Tiles: per kg (0..3): [mu, diag, fac r=0..7]; tile 40 = w."""
    base = K + K * DIM
    blk = DIM + DIM * RANK
    ROW = np.full((NT, 128), -1, np.int64)
    for kg in range(4):
        t0 = kg * 10
        for p in range(128):
            k = 4 * kg + p // 32
            d = p % 32
            ROW[t0 + 0, p] = K + k * DIM + d
            ROW[t0 + 1, p] = base + k * blk + d
            for r in range(RANK):
                ROW[t0 + 2 + r, p] = base + k * blk + DIM + d * RANK + r
    ROW[40, :16] = np.arange(K)
    return ROW


def _prep(inputs):
    import ml_dtypes
    bf = ml_dtypes.bfloat16
    ROW = _feature_rows()
    Wout = np.asarray(inputs["Wout"], np.float32)
    bout = np.asarray(inputs["bout"], np.float32)

    def pack_w(tiles):
        rowsl = ROW[tiles].reshape(-1)
        WP = np.zeros((len(tiles) * 128, H), np.float32)
        v = rowsl >= 0
        WP[v] = Wout[rowsl[v]]
        # [p_h, pr, i, t*128 + p_f] = WP[t*128+p_f, (2*pr+i)*128 + p_h]
        return np.ascontiguousarray(
            WP.T.reshape(2, 2, 128, len(tiles) * 128).transpose(2, 0, 1, 3))

    # bf16 tiles: mu kg (kg*10), diag kg (kg*10+1), w (40)
    bf_tiles = [kg * 10 for kg in range(4)] + \
               [kg * 10 + 1 for kg in range(4)] + [40]
    woutt = pack_w(bf_tiles).astype(bf)
    # fp8 tiles: fac (kg*10+2+r), kg-major then r
    f8_tiles = [kg * 10 + 2 + r for kg in range(4) for r in range(RANK)]
    wout8 = pack_w(f8_tiles).astype(ml_dtypes.float8_e4m3)

    rows = ROW.reshape(-1)
    valid = rows >= 0
    bias_full = np.zeros((NT, 128), np.float32)
    bias_full[valid.reshape(NT, 128)] = bout[rows[valid]]
    biasft = np.zeros((128, 45), np.float32)
    for kg in range(4):
        t0 = kg * 10
        biasft[:, kg] = bias_full[t0 + 0]
        biasft[:, 4 + kg] = bias_full[t0 + 1]
        biasft[:, 8 + kg] = -0.5 * bias_full[t0 + 1]
        for r in range(RANK):
            biasft[:, 12 + kg * 8 + r] = bias_full[t0 + 2 + r]
    biasft[:, 44] = bias_full[40]

    w0t = np.ascontiguousarray(np.asarray(inputs["W0"], np.float32).T).astype(bf)
    wht = np.ascontiguousarray(
        np.transpose(np.asarray(inputs["Wh"], np.float32), (0, 2, 1))).astype(bf)

    def v128(v):
        return np.ascontiguousarray(np.asarray(v, np.float32).reshape(4, 128).T)

    vec_list = [inputs["b0"], inputs["g0"], inputs["be0"]]
    for li in range(NL - 1):
        vec_list += [inputs["bh"][li], inputs["gh"][li], inputs["beh"][li]]
    vecs = np.stack([v128(v) for v in vec_list], axis=-1).astype(np.float32)

    pat8 = np.zeros((128, 8, 32), np.float32)
    for p in range(8):
        for i in range(4):
            pat8[32 * i:32 * (i + 1), p, 4 * p + i] = 1.0
    pat8 = pat8.astype(bf)
    ident = np.eye(128, dtype=np.float32).astype(bf)
    eye16 = np.eye(16, dtype=np.float32)

    data = np.asarray(inputs["data"], np.float32)
    context = np.asarray(inputs["context"], np.float32)
    in_maps = []
    for c in range(N_CORES):
        sl = slice(c * BL, (c + 1) * BL)
        in_maps.append({
            "ctxT": np.ascontiguousarray(context[sl].T).astype(bf),
            "dataT": np.ascontiguousarray(data[sl].T),
            "w0t": w0t, "wht": wht, "woutt": woutt, "wout8": wout8,
            "biasft": biasft,
            "vecs": vecs, "pat8": pat8, "ident": ident, "eye16": eye16,
        })
    return in_maps


def kernel(**inputs):
    from concourse.bass_utils import run_bass_kernel_spmd

    if "nc" not in _CACHE:
        _CACHE["nc"] = build_program()
    nc = _CACHE["nc"]
    in_maps = _prep(inputs)
    res = run_bass_kernel_spmd(nc, in_maps, core_ids=list(range(N_CORES)))
    total = sum(float(res.results[c]["yout"][0, 0]) for c in range(N_CORES))
    return np.float32(-total / B)
